# revision 13
# baseline (speedup 1.0000x reference)
"""Bilateral filter (7x7, sigma_space=5, sigma_color^2*2=0.02) Trainium2 Bass kernel.

Input  I: (8, 1, 480, 640) f32, g: (1, 49, 1, 1) f32 (spatial Gaussian weights)
Output: (8, 480, 640) f32

Sharding: pure data-parallel, one image per NeuronCore (8 cores).

Algorithm (symmetric "t-scheme", validated against the reference in fp64):
For each unordered shift pair {s, -s}, s=(dy,dx), dy>=0 (24 pairs):
    d(p)   = P(p+s) - P(p)                    (P = zero-padded image)
    e(p)   = g_s * exp(-50 d^2)               (shared by both shift directions)
    t(p)   = e(p) * d(p)
    numT  += t(p) - t(p-s)                    (shifted reads via TensorE shift-matmuls)
    den   += e(p) + e(p-s)
den += g_center;  out = P + numT / den        (exactly equals sum(w*Is)/sum(w))

Engine mapping per pair: DVE: sub + mult; ACT: square + exp; TensorE: 4 shift/identity
matmuls accumulating into PSUM (bf16 rhs).  Layout: image rows on partitions,
bands of 122 output rows (+3 halo each side) packed along the free dim.
"""

from contextlib import ExitStack

import numpy as np
import ml_dtypes

import concourse.bass as bass
import concourse.bacc as bacc
import concourse.tile as tile
from concourse import mybir

F32 = mybir.dt.float32
BF16 = mybir.dt.bfloat16

PAD = 3
K = 7
BH = 122          # output rows per band (128 partitions - 2*PAD halo)
SQRT50 = float(np.sqrt(50.0))

FULL_H, FULL_W = 480, 640
N_CORES = 8


def _pairs():
    return [(dy, dx) for dy in range(0, 4) for dx in range(-3, 4)
            if dy > 0 or dx > 0]


def make_wmat():
    """[128, 8, 128] bf16: wmat[:,dy,:] = S_{-dy} (out[m] reads rhs[m-dy]),
    wmat[:,4+dy,:] = -S_{-dy}.  S[k, m] = 1 iff k = m - dy."""
    w = np.zeros((128, 8, 128), np.float32)
    for dy in range(4):
        S = np.diag(np.ones(128 - dy, np.float32), dy)  # [i, i+dy] = 1
        w[:, dy, :] = S
        w[:, 4 + dy, :] = -S
    return w.astype(ml_dtypes.bfloat16)


def _bands(H):
    nb = (H + BH - 1) // BH
    return nb


def _chunks(nbg, Wc):
    """Bank-aligned (<=512 f32, single-PSUM-bank) chunks of the flat
    [nbg*Wc] psum region; returns (band_local, c0, c1) with band-local cols."""
    out = []
    for bl in range(nbg):
        f0, f1 = bl * Wc, (bl + 1) * Wc
        c = f0
        while c < f1:
            nxt = min(f1, (c // 512 + 1) * 512)
            out.append((bl, c - f0, nxt - f0))
            c = nxt
    return out


def _layout(H, W):
    NB = _bands(H)
    PITCH = W + 2 * PAD + (2 * PAD + W) % 2  # even pitch
    return NB, PITCH


def make_imgsh(img, H, W):
    """Host-side layout: [4, 128, NB, PITCH] f32; imgsh[dy, p, b, :] =
    zero-padded image row BH*b + p + dy (padded coords), cols offset PAD."""
    NB, PITCH = _layout(H, W)
    nrow = BH * (NB - 1) + 128 + 3 + 1
    pad2 = np.zeros((nrow, PITCH), np.float32)
    pad2[PAD:PAD + H, PAD:PAD + W] = img
    out = np.empty((4, 128, NB, PITCH), np.float32)
    for dy in range(4):
        for b in range(NB):
            out[dy, :, b, :] = pad2[BH * b + dy: BH * b + dy + 128, :]
    return out


def make_biasv(g49):
    g2 = np.asarray(g49, np.float64).reshape(K, K)
    vals = [float(np.log(g2[PAD + dy, PAD + dx])) for (dy, dx) in _pairs()]
    vals.append(float(g2[PAD, PAD]))
    return np.asarray(vals, np.float32)


def emit(nc: bass.Bass, out_ap: bass.AP, imgsh_ap: bass.AP, wmat_ap: bass.AP,
         biasv_ap: bass.AP, g49: np.ndarray, H: int, W: int, reps: int = 1):
    """Emit the bilateral filter for one [H, W] image."""
    pairs = _pairs()
    NB, PITCH = _layout(H, W)
    groups = []
    b = 0
    while b < NB:
        nbg = min(2, NB - b)
        groups.append((b, nbg))
        b += nbg

    with tile.TileContext(nc) as tc, ExitStack() as ctx:
        singles = ctx.enter_context(tc.tile_pool(name="singles", bufs=1))
        work = ctx.enter_context(tc.tile_pool(name="work", bufs=3))
        epi = ctx.enter_context(tc.tile_pool(name="epi", bufs=2))
        psum = ctx.enter_context(tc.tile_pool(name="psum", bufs=1, space="PSUM"))

        # constants
        wmat = singles.tile([128, 8, 128], BF16)
        nc.sync.dma_start(out=wmat, in_=wmat_ap)

        # per-partition bias constants: ln(g_s) per pair, then g_center;
        # broadcast-DMA the [NPAIR+1] vector to all 128 partitions.
        nbias = len(pairs) + 1
        bias_t = singles.tile([128, nbias], F32)
        nc.sync.dma_start(
            out=bias_t,
            in_=bass.AP(tensor=biasv_ap.tensor, offset=biasv_ap.offset,
                        ap=[[0, 128], [1, nbias]]),
        )

        # Pre-shifted padded image copies (host-laid-out), one per dy in 0..3.
        # Engine APs must start at partition 0/32/64/96, so row (partition)
        # shifts cannot be SBUF views.
        # P_sh[dy] partition p, band b  <=>  padded row BH*b + p + dy.
        P_sh = []
        for dy in range(4):
            Pt = singles.tile([128, NB, PITCH], F32, tag=f"P_sh{dy}")
            nc.sync.dma_start(out=Pt, in_=imgsh_ap[dy])
            P_sh.append(Pt)

        for rep in range(reps):
          for (g0, nbg) in groups:
            num_ps = psum.tile([128, nbg, W], F32)
            den_ps = psum.tile([128, nbg, W], F32)
            chunks = _chunks(nbg, W)
            n_pairs = len(pairs)
            # PSUM bank of each chunk (2048B zero regions; one start/stop per bank)
            bank_of = [((bl * W + c0) * 4) // 2048 for (bl, c0, c1) in chunks]
            first_of_bank = [bank_of.index(b) == i for i, b in enumerate(bank_of)]
            last_of_bank = [
                (len(bank_of) - 1 - bank_of[::-1].index(b)) == i
                for i, b in enumerate(bank_of)
            ]

            for ip, (dy, dx) in enumerate(pairs):
                lo = max(0, -dx)
                wd = 2 * PAD + W - abs(dx)

                d_t = work.tile([128, nbg, PITCH], F32)
                sq_t = work.tile([128, nbg, PITCH], F32)
                e_t = work.tile([128, nbg, PITCH], BF16)
                t_t = work.tile([128, nbg, PITCH], BF16)

                nc.vector.tensor_tensor(
                    out=d_t[:, :, lo:lo + wd],
                    in0=P_sh[dy][:, g0:g0 + nbg, lo + dx:lo + dx + wd],
                    in1=P_sh[0][:, g0:g0 + nbg, lo:lo + wd],
                    op=mybir.AluOpType.subtract,
                )
                nc.scalar.activation(
                    out=sq_t[:, :, lo:lo + wd],
                    in_=d_t[:, :, lo:lo + wd],
                    func=mybir.ActivationFunctionType.Square,
                    bias=0.0,
                    scale=SQRT50,
                )
                nc.scalar.activation(
                    out=e_t[:, :, lo:lo + wd],
                    in_=sq_t[:, :, lo:lo + wd],
                    func=mybir.ActivationFunctionType.Exp,
                    bias=bias_t[:, ip:ip + 1],
                    scale=-1.0,
                )
                nc.vector.tensor_tensor(
                    out=t_t[:, :, lo:lo + wd],
                    in0=e_t[:, :, lo:lo + wd],
                    in1=d_t[:, :, lo:lo + wd],
                    op=mybir.AluOpType.mult,
                )

                first = ip == 0
                last = ip == n_pairs - 1
                # direct: lhsT = Id;  num += t, den += e
                for ic, (bl, c0, c1) in enumerate(chunks):
                    nc.tensor.matmul(
                        num_ps[:, bl, c0:c1],
                        wmat[:, 0, :],
                        t_t[:, bl, PAD + c0:PAD + c1],
                        start=first and first_of_bank[ic], stop=False,
                    )
                for ic, (bl, c0, c1) in enumerate(chunks):
                    nc.tensor.matmul(
                        den_ps[:, bl, c0:c1],
                        wmat[:, 0, :],
                        e_t[:, bl, PAD + c0:PAD + c1],
                        start=first and first_of_bank[ic], stop=False,
                    )
                # view: num -= t(p-s)  (lhsT = -S_{-dy}),  den += e(p-s)
                for ic, (bl, c0, c1) in enumerate(chunks):
                    nc.tensor.matmul(
                        num_ps[:, bl, c0:c1],
                        wmat[:, 4 + dy, :],
                        t_t[:, bl, PAD + c0 - dx:PAD + c1 - dx],
                        start=False, stop=last and last_of_bank[ic],
                    )
                for ic, (bl, c0, c1) in enumerate(chunks):
                    nc.tensor.matmul(
                        den_ps[:, bl, c0:c1],
                        wmat[:, dy, :],
                        e_t[:, bl, PAD + c0 - dx:PAD + c1 - dx],
                        start=False, stop=last and last_of_bank[ic],
                    )

            # epilogue: out = P + numT / (den + g_c)
            deng = epi.tile([128, nbg, W], F32)
            nc.scalar.activation(
                out=deng, in_=den_ps,
                func=mybir.ActivationFunctionType.Identity,
                bias=bias_t[:, len(pairs):],
                scale=1.0,
            )
            r_t = epi.tile([128, nbg, W], F32)
            nc.vector.reciprocal_approx_fast(out=r_t, in_=deng)
            prod = epi.tile([128, nbg, W], F32)
            nc.vector.tensor_tensor(
                out=prod, in0=num_ps, in1=r_t, op=mybir.AluOpType.mult,
            )
            outv = epi.tile([128, nbg, W], F32)
            nc.vector.tensor_tensor(
                out=outv, in0=prod,
                in1=P_sh[0][:, g0:g0 + nbg, PAD:PAD + W],
                op=mybir.AluOpType.add,
            )
            for bl in range(nbg):
                bg = g0 + bl
                rows = min(BH, H - BH * bg)
                nc.sync.dma_start(
                    out=out_ap[BH * bg:BH * bg + rows, 0:W],
                    in_=outv[PAD:PAD + rows, bl, :],
                )


def build_nc(H, W, g49, reps=1):
    nc = bacc.Bacc(num_devices=N_CORES)
    NB, PITCH = _layout(H, W)
    imgsh = nc.dram_tensor("imgsh", [4, 128, NB, PITCH], F32, kind="ExternalInput")
    wmat = nc.dram_tensor("wmat", [128, 8, 128], BF16, kind="ExternalInput")
    biasv = nc.dram_tensor("biasv", [len(_pairs()) + 1], F32, kind="ExternalInput")
    out = nc.dram_tensor("out", [H, W], F32, kind="ExternalOutput")
    emit(nc, out.ap(), imgsh.ap(), wmat.ap(), biasv.ap(), g49, H, W, reps=reps)
    nc.finalize()
    return nc


def kernel(I: np.ndarray, g: np.ndarray) -> np.ndarray:
    from concourse.bass_utils import run_bass_kernel_spmd

    I = np.ascontiguousarray(np.asarray(I, np.float32))
    g49 = np.asarray(g, np.float32).reshape(-1)
    assert I.shape == (N_CORES, 1, FULL_H, FULL_W), I.shape

    nc = build_nc(FULL_H, FULL_W, g49)
    wm = make_wmat()
    bv = make_biasv(g49)
    in_maps = [{"imgsh": make_imgsh(I[c, 0], FULL_H, FULL_W),
                "wmat": wm, "biasv": bv}
               for c in range(N_CORES)]
    res = run_bass_kernel_spmd(nc, in_maps, core_ids=list(range(N_CORES)))
    global LAST_RESULTS
    LAST_RESULTS = res
    return np.stack([r["out"] for r in res.results], axis=0)


LAST_RESULTS = None


if __name__ == "__main__":
    rng = np.random.default_rng(0)
    I = rng.random((N_CORES, 1, FULL_H, FULL_W), dtype=np.float32)
    g2 = np.exp(-(np.arange(-3, 4)[None, :] ** 2 + np.arange(-3, 4)[:, None] ** 2) / 50.0)
    out = kernel(I, g2.reshape(1, 49, 1, 1).astype(np.float32))
    print(out.shape, out.dtype)


# revision 14
# speedup vs baseline: 3.8586x; 3.8586x over previous
"""Bilateral filter v2 — instruction-minimized for the axon-trn2 platform.

Asymmetric 49-shift scheme; the 7 dx-shifts of each dy are batched into single
ops via windowed (overlapping) access patterns on a pad-3 image layout.
No TensorE, no PSUM.  out = num/den directly.
"""
from contextlib import ExitStack

import numpy as np
import ml_dtypes

import concourse.bass as bass
import concourse.bacc as bacc
import concourse.tile as tile
from concourse import mybir

F32 = mybir.dt.float32
BF16 = mybir.dt.bfloat16

PAD = 3
K = 7
BH = 122
SQRT50 = float(np.sqrt(50.0))
FULL_H, FULL_W = 480, 640
N_CORES = 8


def _bands(H):
    return (H + BH - 1) // BH


def _layout(H, W):
    NB = _bands(H)
    PITCH = W + 2 * PAD  # 646
    assert NB % 2 == 0, "grouping assumes an even number of bands"
    return NB, PITCH


def make_imgsh2(img, H, W):
    """Returns (f32 scaled by sqrt50, bf16 unscaled), both
    [NGRP, 128, 7(jdy), 2*PITCH] laid out per 2-band group:
    value(p, jdy, bl*PITCH + c) = pad(BH*(2g+bl) + p + jdy - 3, c)."""
    NB, PITCH = _layout(H, W)
    ngrp = NB // 2
    nrow = 3 + BH * (NB - 1) + 128 + 8
    pad2 = np.zeros((nrow, PITCH), np.float32)
    pad2[3 + PAD:3 + PAD + H, PAD:PAD + W] = img
    out = np.empty((ngrp, 128, 7, 2 * PITCH), np.float32)
    for g in range(ngrp):
        for j in range(7):
            for bl in range(2):
                r0 = 3 + BH * (2 * g + bl) + (j - 3)
                out[g, :, j, bl * PITCH:(bl + 1) * PITCH] = pad2[r0:r0 + 128, :]
    return ((out * SQRT50).astype(np.float32), out.astype(ml_dtypes.bfloat16))


def make_bias2(g49):
    g2 = np.asarray(g49, np.float64).reshape(K, K)
    v = np.zeros(14, np.float32)
    for j in range(7):
        v[j] = np.log(g2[j, PAD])          # -dy^2/50 (exp bias)
        v[7 + j] = -np.log(g2[PAD, j])     # +dx^2/50 (added to A)
    return v


def _win(tile_obj, part_ap, off, dims):
    base = part_ap
    return bass.AP(tensor=base.tensor, offset=base.offset + off, ap=[base.ap[0]] + dims)


def emit2(nc, out_ap, imgsh_ap, bias_ap, H, W, reps=1):
    NB, PITCH = _layout(H, W)
    ngrp = NB // 2
    nbg = 2

    with tile.TileContext(nc) as tc, ExitStack() as ctx:
        singles = ctx.enter_context(tc.tile_pool(name="singles", bufs=1))
        gpool = ctx.enter_context(tc.tile_pool(name="gpool", bufs=1))
        apool = ctx.enter_context(tc.tile_pool(name="apool", bufs=2))
        wpool = ctx.enter_context(tc.tile_pool(name="wpool", bufs=1))

        bias_t = singles.tile([128, 14], F32)
        nc.sync.dma_start(
            out=bias_t,
            in_=bass.AP(tensor=bias_ap.tensor, offset=bias_ap.offset,
                        ap=[[0, 128], [1, 14]]),
        )

        for rep in range(reps):
         for g in range(ngrp):
            ps = gpool.tile([128, 7, nbg * PITCH], F32, name="ps")
            nc.sync.dma_start(out=ps, in_=imgsh_ap[g])

            nd = gpool.tile([128, 2, nbg, W], F32, name="nd")
            tmp = gpool.tile([128, 2, nbg, W], F32, name="tmp")

            ps0 = ps[:, 0, 0:1]        # AP carrying the partition dim entry

            for jdy in range(7):
                A = apool.tile([128, nbg, 7, W], F32)
                Wt = wpool.tile([128, nbg, 7, W], BF16, name="Wt")
                Ut = wpool.tile([128, nbg, 7, W], F32, name="Ut")
                wt0 = Wt[:, 0, 0, 0:1]
                ut0 = Ut[:, 0, 0, 0:1]
                for bl in range(nbg):
                    nc.vector.tensor_tensor(
                        out=A[:, bl],
                        in0=_win(ps, ps0, jdy * nbg * PITCH + bl * PITCH,
                                 [[1, 7], [1, W]]),
                        in1=_win(ps, ps0, 3 * nbg * PITCH + bl * PITCH + PAD,
                                 [[0, 7], [1, W]]),
                        op=mybir.AluOpType.subtract,
                    )
                Afl = A[:].rearrange("p a b c -> p (a b c)")
                nc.scalar.activation(
                    out=Afl, in_=Afl,
                    func=mybir.ActivationFunctionType.Square,
                    bias=0.0, scale=1.0,
                )
                for bl in range(nbg):
                    nc.vector.scalar_tensor_tensor(
                        out=A[:, bl], in0=A[:, bl], scalar=1.0,
                        in1=bass.AP(tensor=bias_t[:].tensor,
                                    offset=bias_t[:].offset + 7,
                                    ap=[bias_t[:].ap[0], [1, 7], [0, W]]),
                        op0=mybir.AluOpType.mult,
                        op1=mybir.AluOpType.add,
                    )
                Wfl = _win(Wt, wt0, 0, [[1, nbg * 7 * W]])
                nc.scalar.activation(
                    out=Wfl, in_=Afl,
                    func=mybir.ActivationFunctionType.Exp,
                    bias=bias_t[:, jdy:jdy + 1], scale=-1.0,
                )
                for bl in range(nbg):
                    nc.vector.tensor_tensor(
                        out=Ut[:, bl],
                        in0=Wt[:, bl],
                        in1=_win(ps, ps0, jdy * nbg * PITCH + bl * PITCH,
                                 [[1, 7], [1, W]]),
                        op=mybir.AluOpType.mult,
                    )
                dst = nd if jdy == 0 else tmp
                for wu in range(2):
                    src_t, s0ap = (Wt, wt0) if wu == 0 else (Ut, ut0)
                    for bl in range(nbg):
                        nc.vector.tensor_reduce(
                            out=dst[:, wu, bl, :],
                            in_=_win(src_t, s0ap, bl * 7 * W,
                                     [[1, W], [W, 7]]),
                            axis=mybir.AxisListType.X,
                            op=mybir.AluOpType.add,
                        )
                if jdy > 0:
                    nc.vector.tensor_tensor(
                        out=nd[:].rearrange("p a b c -> p (a b c)"),
                        in0=nd[:].rearrange("p a b c -> p (a b c)"),
                        in1=tmp[:].rearrange("p a b c -> p (a b c)"),
                        op=mybir.AluOpType.add)

            r = tmp[:, 0].rearrange("p a b -> p (a b)")
            nc.vector.reciprocal_approx_fast(
                out=r, in_=nd[:, 0].rearrange("p a b -> p (a b)"))
            outv = tmp[:, 1]
            nc.vector.scalar_tensor_tensor(
                out=outv.rearrange("p a b -> p (a b)"),
                in0=nd[:, 1].rearrange("p a b -> p (a b)"),
                scalar=1.0 / SQRT50,
                in1=r,
                op0=mybir.AluOpType.mult,
                op1=mybir.AluOpType.mult)
            for bl in range(nbg):
                bg = 2 * g + bl
                rows = min(BH, H - BH * bg)
                nc.sync.dma_start(
                    out=out_ap[BH * bg:BH * bg + rows, 0:W],
                    in_=outv[PAD:PAD + rows, bl, :],
                )


def build_nc2(H, W, g49, reps=1):
    nc = bacc.Bacc(num_devices=N_CORES)
    NB, PITCH = _layout(H, W)
    ngrp = NB // 2
    imgsh = nc.dram_tensor("imgsh", [ngrp, 128, 7, 2 * PITCH], F32,
                           kind="ExternalInput")
    biasv = nc.dram_tensor("biasv", [14], F32, kind="ExternalInput")
    out = nc.dram_tensor("out", [H, W], F32, kind="ExternalOutput")
    emit2(nc, out.ap(), imgsh.ap(), biasv.ap(), H, W, reps=reps)
    nc.finalize()
    return nc


def kernel(I: np.ndarray, g: np.ndarray) -> np.ndarray:
    from concourse.bass_utils import run_bass_kernel_spmd

    I = np.ascontiguousarray(np.asarray(I, np.float32))
    g49 = np.asarray(g, np.float32).reshape(-1)
    nc = build_nc2(FULL_H, FULL_W, g49)
    bv = make_bias2(g49)
    in_maps = []
    for c in range(N_CORES):
        a, _ = make_imgsh2(I[c, 0], FULL_H, FULL_W)
        in_maps.append({"imgsh": a, "biasv": bv})
    res = run_bass_kernel_spmd(nc, in_maps, core_ids=list(range(N_CORES)))
    global LAST_RESULTS
    LAST_RESULTS = res
    return np.stack([r["out"] for r in res.results], axis=0)


LAST_RESULTS = None
